# revision 11
# baseline (speedup 1.0000x reference)
"""CompGCN (2-layer) Trainium2 kernel, 8-core SPMD.

Strategy
--------
Algebra: per conv layer, only the opt_e / ipt_e edge modes reach the output
(r-mode scatter rows are all < num_ent and the relation stream reads rows
>= num_ent), and (x[col] @ W) * nrm scattered to rows == S @ W with
S[r] = sum_e nrm_e * x[col_e].  The per-mode weight and the ent_w block
matmul fold into one 256x256 matrix per mode (host constant folding):

  x_ent' = tanh(tanh( S_opt @ A + x_ent @ B + S_ipt @ C + ent_b ))
  x_rel' = tanh(tanh( x_rel @ R + rel_b ))
  out    = x0 @ F0 + x1 @ F1 + x2 @ F2 + final_b

Sharding: entity rows padded to 100352 = 784 tiles of 128, split
contiguously across 8 cores (98 tiles each).  Each core computes its own
row-tiles end to end; the relation block (500 rows -> 4 tiles) is computed
redundantly on every core.  No collectives: the host regathers the edge
messages (and retransposes activations) between the two launches.

Device compute per tile: S^T accumulated in PSUM by matmuls
  S^T[f, r'] += G_chunk[128e, 128f].T @ M[128e, 256r']
where G_chunk holds pregathered source features (128 edges of this tile,
both modes mixed) and M[e, r'] = (iota256[e,:] == lr'[e]) * nrm[e] is built
on the vector engine (r' = dest partition + 128 * is_ipt).

Launch A (conv layer 1): dense matmuls row-major (lhsT = S^T / x^T chunks,
rhs = folded weights), bias as a rank-1 matmul, tanh twice on ACT, store.

Launch B (conv layer 2 + final): the conv dense runs TRANSPOSED
(lhsT = weight blocks, rhs = S^T / x^T chunks) so x2^T materializes
directly in SBUF, and the final projection fuses into the same tile
iteration - no DRAM round trip, no xbar transposes, no phase barrier.

DMA issue is spread across both HWDGE sequencers (SP + ACT) with stores on
gpsimd (SWDGE); loads are batched over pairs of row tiles.  bf16
activations/weights, fp32 PSUM accumulation.
"""
import sys

sys.path.insert(0, "/opt/trn_rl_repo")

import numpy as np
import ml_dtypes

from concourse import bass, mybir, bacc, tile, bass_utils

bf16 = ml_dtypes.bfloat16

NCORES = 8
P = 128
D = 256
NUM_ENT = 100000
NUM_REL = 500
ENT_PAD = 100352            # 784 tiles
TILES = ENT_PAD // P        # 784
TPC = TILES // NCORES       # 98 tiles per core
RPC = ENT_PAD // NCORES     # 12544 rows per core
REL_PAD = 512
RT = REL_PAD // P           # 4 relation tiles

_prog_cache = {}


def _build_program(Ko, Ki, final, use_bias):
    """One conv layer (final=False) or conv2+final fused (final=True)."""
    K = [a + b for a, b in zip(Ko, Ki)]
    C_total = int(sum(K))
    nc = bacc.Bacc("TRN2", target_bir_lowering=False, debug=False,
                   num_devices=NCORES)
    dt = mybir.dt
    T = lambda name, shape, d, kind: nc.dram_tensor(name, shape, d, kind=kind).ap()

    GATH = T("GATH", [P, C_total * D], dt.bfloat16, "ExternalInput")
    LR = T("LR", [P, C_total], dt.float32, "ExternalInput")
    NR = T("NR", [P, C_total], dt.float32, "ExternalInput")
    XPT = T("XPT", [P, 2, RPC], dt.bfloat16, "ExternalInput")
    XRPT = T("XRPT", [P, 2, REL_PAD], dt.bfloat16, "ExternalInput")
    WA = T("WA", [P, 2 * D], dt.bfloat16, "ExternalInput")
    WB = T("WB", [P, 2 * D], dt.bfloat16, "ExternalInput")
    WC = T("WC", [P, 2 * D], dt.bfloat16, "ExternalInput")
    WR = T("WR", [P, 2 * D], dt.bfloat16, "ExternalInput")
    EB = T("EB", [1, D], dt.bfloat16, "ExternalInput")
    RB = T("RB", [1, D], dt.bfloat16, "ExternalInput")
    IOTA = T("IOTA", [P, 2 * P], dt.bfloat16, "ExternalInput")
    ONES = T("ONES", [1, P], dt.bfloat16, "ExternalInput")
    if not final:
        XE_OUT = T("XE_OUT", [RPC, D], dt.bfloat16, "ExternalOutput")
        XR_OUT = T("XR_OUT", [REL_PAD, D], dt.bfloat16, "ExternalOutput")
    else:
        X0T = T("X0T", [P, 2, RPC], dt.bfloat16, "ExternalInput")
        X0RT = T("X0RT", [P, 2, REL_PAD], dt.bfloat16, "ExternalInput")
        WF0 = T("WF0", [P, 2 * D], dt.bfloat16, "ExternalInput")
        WF1 = T("WF1", [P, 2 * D], dt.bfloat16, "ExternalInput")
        WF2 = T("WF2", [P, 2 * D], dt.bfloat16, "ExternalInput")
        FB = T("FB", [1, D], dt.bfloat16, "ExternalInput")
        OUT_E = T("OUT_E", [RPC, D], dt.float32, "ExternalOutput")
        OUT_R = T("OUT_R", [REL_PAD, D], dt.float32, "ExternalOutput")

    off = np.concatenate([[0], np.cumsum(K)]).astype(int)
    Tanh = mybir.ActivationFunctionType.Tanh

    with tile.TileContext(nc) as tc:
        with (
            tc.tile_pool(name="consts", bufs=1) as cp,
            tc.tile_pool(name="work", bufs=3) as wp,
            tc.tile_pool(name="mpool", bufs=1) as mp,
            tc.tile_pool(name="psS", bufs=2, space="PSUM") as psS,
            tc.tile_pool(name="psE", bufs=2, space="PSUM") as psE,
            tc.tile_pool(name="psT", bufs=1, space="PSUM") as psT,
        ):
            # resident constants
            lr_sb = cp.tile([P, C_total], dt.float32, tag="lr")
            nc.sync.dma_start(lr_sb[:], LR[:])
            nr_sb = cp.tile([P, C_total], dt.float32, tag="nr")
            nc.sync.dma_start(nr_sb[:], NR[:])
            iota_sb = cp.tile([P, 2 * P], dt.bfloat16, tag="iota")
            nc.sync.dma_start(iota_sb[:], IOTA[:])
            ones_sb = cp.tile([1, P], dt.bfloat16, tag="ones")
            nc.sync.dma_start(ones_sb[:], ONES[:])

            def wtile(ap, name):
                t = cp.tile([P, 2 * D], dt.bfloat16, tag=f"w_{name}")
                nc.sync.dma_start(t[:], ap[:])
                return t

            wa, wb, wc, wr = (wtile(WA, "a"), wtile(WB, "b"),
                              wtile(WC, "c"), wtile(WR, "r"))

            def btile(ap, name):
                t = cp.tile([1, D], dt.bfloat16, tag=f"b_{name}")
                nc.sync.dma_start(t[:], ap[:])
                return t

            eb, rb = btile(EB, "e"), btile(RB, "r")
            if not use_bias:
                eb = rb = None
                fbN = None
            if final:
                wf0, wf1, wf2 = wtile(WF0, "f0"), wtile(WF1, "f1"), wtile(WF2, "f2")
                fb = btile(FB, "f") if use_bias else None

            def load_xt_pair(src, r0, n, tag, eng):
                xt = wp.tile([P, 2, n], dt.bfloat16, tag=tag)
                eng.dma_start(xt[:], src[:, :, r0:r0 + n])
                return xt

            def scatter_ST(t, g, ob):
                """Accumulate S^T for tile slot t from gathered chunk block g.
                S0/S1 hold [S_opt^T | S_ipt^T] for f-chunk 0/1; opt chunks
                accumulate into cols 0:128, ipt chunks into cols 128:256
                (disjoint PSUM regions, separate accumulation groups)."""
                Kot, Kit = int(Ko[t]), int(Ki[t])
                S0 = psS.tile([P, 2 * P], dt.float32, space="PSUM", tag="S0")
                S1 = psS.tile([P, 2 * P], dt.float32, space="PSUM", tag="S1")
                ms = []
                for c in range(Kot + Kit):
                    m = mp.tile([P, P], dt.bfloat16, tag=f"m{c % 8}")
                    oc = int(off[t]) + c
                    nc.vector.tensor_scalar(
                        out=m[:], in0=iota_sb[:, 0:P],
                        scalar1=lr_sb[:, oc:oc + 1],
                        scalar2=nr_sb[:, oc:oc + 1],
                        op0=mybir.AluOpType.is_equal, op1=mybir.AluOpType.mult,
                    )
                    ms.append(m)
                for mode0, kn in ((0, Kot), (1, Kit)):
                    lo = mode0 * Kot
                    csl = slice(mode0 * P, (mode0 + 1) * P)
                    for S, fo in ((S0, 0), (S1, P)):
                        for c in range(kn):
                            cb = (ob + lo + c) * D + fo
                            nc.tensor.matmul(
                                S[:, csl], lhsT=g[:, cb:cb + P], rhs=ms[lo + c][:],
                                start=(c == 0), stop=(c == kn - 1),
                                skip_group_check=True)
                sb0 = wp.tile([P, 2 * P], dt.bfloat16, tag="sb0")
                nc.vector.tensor_copy(sb0[:], S0[:])
                sb1 = wp.tile([P, 2 * P], dt.bfloat16, tag="sb1")
                nc.vector.tensor_copy(sb1[:], S1[:])
                return sb0, sb1

            def dense_row(terms, bias_sb, out_ap, r0, act2, eng_store):
                """Row-major dense: out[128r, 256] in one PSUM tile."""
                e = psE.tile([P, D], dt.float32, space="PSUM", tag="eout")
                mms = []
                for pair, w in terms:
                    mms.append((pair[0], w[:, 0:D]))
                    mms.append((pair[1], w[:, D:2 * D]))
                if bias_sb is not None:
                    mms.append((ones_sb[0:1, :], bias_sb[0:1, :]))
                for i, (l, r) in enumerate(mms):
                    nc.tensor.matmul(e[:], lhsT=l, rhs=r,
                                     start=(i == 0), stop=(i == len(mms) - 1))
                if act2:
                    t1 = wp.tile([P, D], dt.bfloat16, tag="t1")
                    nc.scalar.activation(t1[:], e[:], Tanh)
                    o = wp.tile([P, D], dt.bfloat16, tag="t2")
                    nc.scalar.activation(o[:], t1[:], Tanh)
                else:
                    o = wp.tile([P, D], dt.float32, tag="outf")
                    nc.vector.tensor_copy(o[:], e[:])
                eng_store.dma_start(out_ap[r0:r0 + P, :], o[:])
                return o

            def dense_T_tanh2(rhs_terms, bias_sb, x2t_tag):
                """Transposed dense -> x2^T [128f, 256r-of...]: out[:, j*128:] =
                tanh2( sum_k W[k,j].T-block @ rhs_k ).  rhs_terms: list of
                (w_tile, (rhs_k0, rhs_k1)); weight tile packed [128, 4*128]
                with block (2k+j)."""
                x2t = wp.tile([P, 2 * P], dt.bfloat16, tag=x2t_tag)
                for jj in (0, 1):
                    eT = psT.tile([P, P], dt.float32, space="PSUM", tag=f"eT{jj}")
                    mms = []
                    for w, rhs in rhs_terms:
                        for k in (0, 1):
                            blk = (2 * k + jj) * P
                            mms.append((w[:, blk:blk + P], rhs[k]))
                    if bias_sb is not None:
                        mms.append((bias_sb[0:1, jj * P:(jj + 1) * P], ones_sb[0:1, :]))
                    for i, (l, r) in enumerate(mms):
                        nc.tensor.matmul(eT[:], lhsT=l, rhs=r,
                                         start=(i == 0), stop=(i == len(mms) - 1))
                    ta = wp.tile([P, P], dt.bfloat16, tag=f"ta{jj}")
                    nc.scalar.activation(ta[:], eT[:], Tanh)
                    nc.scalar.activation(x2t[:, jj * P:(jj + 1) * P], ta[:], Tanh)
                return x2t

            # ================= entity tile pairs =================
            for pt in range(TPC // 2):
                t0 = 2 * pt
                o0, o2 = int(off[t0]), int(off[t0 + 2])
                g = wp.tile([P, (o2 - o0) * D], dt.bfloat16, tag="g")
                nc.sync.dma_start(g[:], GATH[:, o0 * D:o2 * D])
                xt = load_xt_pair(XPT, t0 * P, 2 * P, "xt", nc.scalar)
                if final:
                    xt0 = load_xt_pair(X0T, t0 * P, 2 * P, "xt0", nc.sync)
                for u in (0, 1):
                    t = t0 + u
                    usl = slice(u * P, (u + 1) * P)
                    sb0, sb1 = scatter_ST(t, g, int(off[t]) - o0)
                    if not final:
                        dense_row(
                            [((sb0[:, 0:P], sb1[:, 0:P]), wa),
                             ((xt[:, 0, usl], xt[:, 1, usl]), wb),
                             ((sb0[:, P:2 * P], sb1[:, P:2 * P]), wc)],
                            eb, XE_OUT, t * P, act2=True, eng_store=nc.gpsimd)
                    else:
                        x2t = dense_T_tanh2(
                            [(wa, (sb0[:, 0:P], sb1[:, 0:P])),
                             (wb, (xt[:, 0, usl], xt[:, 1, usl])),
                             (wc, (sb0[:, P:2 * P], sb1[:, P:2 * P]))],
                            eb, "x2t")
                        dense_row(
                            [((xt0[:, 0, usl], xt0[:, 1, usl]), wf0),
                             ((xt[:, 0, usl], xt[:, 1, usl]), wf1),
                             ((x2t[:, 0:P], x2t[:, P:2 * P]), wf2)],
                            fb, OUT_E, t * P, act2=False, eng_store=nc.gpsimd)

            # ================= relation tile pairs =================
            for pt in range(RT // 2):
                t0 = 2 * pt
                xtr = load_xt_pair(XRPT, t0 * P, 2 * P, "xtr", nc.scalar)
                if final:
                    xtr0 = load_xt_pair(X0RT, t0 * P, 2 * P, "xtr0", nc.sync)
                for u in (0, 1):
                    t = t0 + u
                    usl = slice(u * P, (u + 1) * P)
                    if not final:
                        dense_row(
                            [((xtr[:, 0, usl], xtr[:, 1, usl]), wr)],
                            rb, XR_OUT, t * P, act2=True, eng_store=nc.gpsimd)
                    else:
                        x2rt = dense_T_tanh2(
                            [(wr, (xtr[:, 0, usl], xtr[:, 1, usl]))], rb, "x2rt")
                        dense_row(
                            [((xtr0[:, 0, usl], xtr0[:, 1, usl]), wf0),
                             ((xtr[:, 0, usl], xtr[:, 1, usl]), wf1),
                             ((x2rt[:, 0:P], x2rt[:, P:2 * P]), wf2)],
                            fb, OUT_R, t * P, act2=False, eng_store=nc.gpsimd)
    nc.compile()
    return nc


def _get_program(Ko, Ki, final, use_bias):
    key = (tuple(Ko), tuple(Ki), final, use_bias)
    if key not in _prog_cache:
        _prog_cache[key] = _build_program(Ko, Ki, final, use_bias)
    return _prog_cache[key]


def _pack_w(w):
    """[256, 256] -> [128, 512]; [:, :256] = rows 0:128, [:, 256:] = rows 128:."""
    w = np.asarray(w, np.float32)
    return w.reshape(2, P, D).transpose(1, 0, 2).reshape(P, 2 * D).astype(bf16)


def _pack_wT(w):
    """[256, 256] -> [128, 4*128] blocks (2k+j) = W[k*128:(k+1)*128, j*128:(j+1)*128]."""
    w = np.asarray(w, np.float32)
    return np.ascontiguousarray(
        w.reshape(2, P, 2, P).transpose(1, 0, 2, 3).reshape(P, 4 * P)).astype(bf16)


def _transpose_feats(x):
    """[N, 256] -> [128, 2, N] with [p, k, r] = x[r, k*128+p]."""
    n = x.shape[0]
    return np.ascontiguousarray(np.asarray(x).reshape(n, 2, P).transpose(2, 1, 0))


def _inv_sqrt_deg(idx, n):
    deg = np.bincount(idx, minlength=n).astype(np.float32)
    with np.errstate(divide="ignore"):
        return np.where(deg > 0, deg ** -0.5, 0.0).astype(np.float32)


def kernel(x, edge_index, num_ent, num_rel, params):
    num_ent = int(num_ent)
    num_rel = int(num_rel)
    assert num_ent == NUM_ENT and num_rel == NUM_REL
    x = np.asarray(x, np.float32)
    ei = np.asarray(edge_index).astype(np.int64)
    E = ei.shape[1]
    a_, b_ = E // 3, (2 * E) // 3
    num_nodes = num_ent + num_rel

    # --- per-edge norms (opt_e and ipt_e modes only; r-modes are dead) ---
    rows = np.concatenate([ei[0, :a_], ei[0, a_:b_]])
    cols = np.concatenate([ei[1, :a_], ei[1, a_:b_]])
    nrms = []
    for lo, hi in ((0, a_), (a_, b_)):
        r, c = ei[0, lo:hi], ei[1, lo:hi]
        nrms.append(_inv_sqrt_deg(r, num_ent)[r] * _inv_sqrt_deg(c, num_nodes)[c])
    nrm = np.concatenate(nrms)
    is_ipt = np.zeros(rows.shape[0], np.int64)
    is_ipt[a_:] = 1

    # --- pack rows into tiles (snake deal by degree -> ~equal edges/tile) ---
    deg = np.bincount(rows, minlength=ENT_PAD)
    srt = np.argsort(-deg, kind="stable")
    snake = srt.reshape(P, TILES).copy()        # [round p, bin]
    snake[1::2] = snake[1::2, ::-1]
    bin_cnt = deg[snake].sum(0)
    brank = np.argsort(-bin_cnt, kind="stable")  # bins sorted by load desc
    # bin with rank r -> core r % 8, slot r // 8; device row = snake[p, bin]
    perm = np.empty(ENT_PAD, np.int64)           # device idx -> global row
    rr = np.arange(TILES)
    core_b = np.empty(TILES, np.int64); slot_b = np.empty(TILES, np.int64)
    core_b[brank] = rr % NCORES
    slot_b[brank] = rr // NCORES
    dev_base = core_b * RPC + slot_b * P         # per bin
    for p in range(P):
        perm[dev_base + p] = snake[p]
    inv = np.empty(ENT_PAD, np.int64)
    inv[perm] = np.arange(ENT_PAD)

    dev_idx = inv[rows]
    tile_dev = dev_idx // P                      # core-major tile id
    key2 = tile_dev * 2 + is_ipt                 # (tile, mode) segments
    order = np.argsort(key2, kind="stable")
    cols_s, nrm_s, ipt_s = cols[order], nrm[order], is_ipt[order]
    pp_s = (dev_idx % P)[order]
    cnt2 = np.bincount(key2, minlength=2 * TILES).reshape(TILES, 2)
    # core-major: tile_dev = core * TPC + slot
    Ko = np.maximum(1, -(-cnt2[:, 0].reshape(NCORES, TPC).max(0) // P))
    Ki = np.maximum(1, -(-cnt2[:, 1].reshape(NCORES, TPC).max(0) // P))
    K = Ko + Ki
    off = np.concatenate([[0], np.cumsum(K)]).astype(int)
    C_total = int(off[-1])

    tstart = np.concatenate([[0], np.cumsum(cnt2.ravel())]).astype(int)
    j = np.arange(dev_idx.shape[0]) - tstart[key2[order]]

    lr_all = np.zeros((NCORES, C_total, P), np.float32)
    nr_all = np.zeros((NCORES, C_total, P), np.float32)
    col_all = np.zeros((NCORES, C_total, P), np.int64)
    core_s = tile_dev[order] // TPC
    slot_s = tile_dev[order] % TPC
    cpos = off[slot_s] + ipt_s * Ko[slot_s] + j // P
    ppos = j % P
    lr_all[core_s, cpos, ppos] = pp_s.astype(np.float32)
    nr_all[core_s, cpos, ppos] = nrm_s
    col_all[core_s, cpos, ppos] = cols_s

    # --- folded weights ---
    layers = params["layers"]

    def fold(lp, packer):
        ew = np.asarray(lp["ent_w"], np.float32)
        rw = np.asarray(lp["rel_w"], np.float32)
        return dict(
            WA=packer(np.asarray(lp["w_opt_e"], np.float32) @ ew[0:D]),
            WB=packer(np.asarray(lp["w_loop_e"], np.float32) @ ew[D:2 * D]),
            WC=packer(np.asarray(lp["w_ipt_e"], np.float32) @ ew[2 * D:3 * D]),
            WR=packer(np.asarray(lp["w_loop_r"], np.float32) @ rw[D:2 * D]),
            EB=np.asarray(lp["ent_b"], np.float32).reshape(1, D).astype(bf16),
            RB=np.asarray(lp["rel_b"], np.float32).reshape(1, D).astype(bf16),
        )

    W1 = fold(layers[0], _pack_w)
    W2 = fold(layers[1], _pack_wT)          # launch B conv runs transposed
    Fw = np.asarray(params["final_w"], np.float32)
    WF = dict(WF0=_pack_w(Fw[0:D]), WF1=_pack_w(Fw[D:2 * D]),
              WF2=_pack_w(Fw[2 * D:3 * D]),
              FB=np.asarray(params["final_b"], np.float32).reshape(1, D).astype(bf16))

    bias_vals = [W1['EB'], W1['RB'], W2['EB'], W2['RB'], WF['FB']]
    iota = np.broadcast_to(np.arange(2 * P, dtype=np.float32), (P, 2 * P)).astype(bf16)
    ones = np.ones((1, P), bf16)

    x0e = np.zeros((ENT_PAD, D), bf16)
    x0e[:num_ent] = x[:num_ent].astype(bf16)
    x0r = np.zeros((REL_PAD, D), bf16)
    x0r[:num_rel] = x[num_ent:].astype(bf16)

    def gath(xsrc):
        g = xsrc[col_all]                      # [NCORES, C_total, P, D]
        return np.ascontiguousarray(g.transpose(0, 2, 1, 3)).reshape(NCORES, P, C_total * D)

    def in_maps(xe_prev_T, xr_prev_T, wts, g_all, extra=None):
        maps = []
        for k in range(NCORES):
            m = dict(
                GATH=g_all[k],
                LR=np.ascontiguousarray(lr_all[k].T),
                NR=np.ascontiguousarray(nr_all[k].T),
                XPT=xe_prev_T[k], XRPT=xr_prev_T,
                IOTA=iota, ONES=ones, **wts,
            )
            if extra:
                m.update({kk: (vv[k] if isinstance(vv, list) else vv)
                          for kk, vv in extra.items()})
            maps.append(m)
        return maps

    exec_ns = []

    def run(nc, maps, trace=False):
        res = bass_utils.run_bass_kernel_spmd(nc, maps, core_ids=list(range(NCORES)),
                                              trace=trace)
        if res.exec_time_ns:
            exec_ns.append(res.exec_time_ns)
        return res.results

    trace = bool(getattr(kernel, "trace", False))

    x0e_dev = x0e[perm]
    x0eT = [_transpose_feats(x0e_dev[k * RPC:(k + 1) * RPC]) for k in range(NCORES)]
    x0rT = _transpose_feats(x0r)

    # ---- launch A: conv layer 1 ----
    use_bias = bool(any(np.any(np.asarray(v, np.float32)) for v in bias_vals))
    ncA = _get_program(Ko, Ki, final=False, use_bias=use_bias)
    resA = run(ncA, in_maps(x0eT, x0rT, W1, gath(x0e)), trace)
    x1e_dev = np.concatenate([resA[k]["XE_OUT"] for k in range(NCORES)], 0)
    x1r = resA[0]["XR_OUT"]
    x1e = np.empty_like(x1e_dev)
    x1e[perm] = x1e_dev

    # ---- launch B: conv layer 2 + final projection (fused) ----
    ncB = _get_program(Ko, Ki, final=True, use_bias=use_bias)
    x1eT = [_transpose_feats(x1e_dev[k * RPC:(k + 1) * RPC]) for k in range(NCORES)]
    extra = {"X0T": x0eT, "X0RT": x0rT, **WF}
    resB = run(ncB, in_maps(x1eT, _transpose_feats(x1r), W2, gath(x1e), extra), trace)

    oe_dev = np.concatenate([resB[k]["OUT_E"] for k in range(NCORES)], 0)
    oe = np.empty_like(oe_dev)
    oe[perm] = oe_dev
    out = np.empty((num_nodes, D), np.float32)
    out[:num_ent] = oe[:num_ent]
    out[num_ent:] = resB[0]["OUT_R"][:num_rel]
    kernel.last_exec_ns = exec_ns
    return out


# revision 12
# speedup vs baseline: 1.0006x; 1.0006x over previous
"""CompGCN (2-layer) Trainium2 kernel, 8-core SPMD.

Strategy
--------
Algebra: per conv layer, only the opt_e / ipt_e edge modes reach the output
(r-mode scatter rows are all < num_ent and the relation stream reads rows
>= num_ent), and (x[col] @ W) * nrm scattered to rows == S @ W with
S[r] = sum_e nrm_e * x[col_e].  The per-mode weight and the ent_w block
matmul fold into one 256x256 matrix per mode (host constant folding):

  x_ent' = tanh(tanh( S_opt @ A + x_ent @ B + S_ipt @ C + ent_b ))
  x_rel' = tanh(tanh( x_rel @ R + rel_b ))
  out    = x0 @ F0 + x1 @ F1 + x2 @ F2 + final_b

Sharding: entity rows padded to 100352 = 784 tiles of 128, split
contiguously across 8 cores (98 tiles each).  Each core computes its own
row-tiles end to end; the relation block (500 rows -> 4 tiles) is computed
redundantly on every core.  No collectives: the host regathers the edge
messages (and retransposes activations) between the two launches.

Device compute per tile: S^T accumulated in PSUM by matmuls
  S^T[f, r'] += G_chunk[128e, 128f].T @ M[128e, 256r']
where G_chunk holds pregathered source features (128 edges of this tile,
both modes mixed) and M[e, r'] = (iota256[e,:] == lr'[e]) * nrm[e] is built
on the vector engine (r' = dest partition + 128 * is_ipt).

Launch A (conv layer 1): dense matmuls row-major (lhsT = S^T / x^T chunks,
rhs = folded weights), bias as a rank-1 matmul, tanh twice on ACT, store.

Launch B (conv layer 2 + final): the conv dense runs TRANSPOSED
(lhsT = weight blocks, rhs = S^T / x^T chunks) so x2^T materializes
directly in SBUF, and the final projection fuses into the same tile
iteration - no DRAM round trip, no xbar transposes, no phase barrier.

DMA issue is spread across both HWDGE sequencers (SP + ACT) with stores on
gpsimd (SWDGE); loads are batched over pairs of row tiles.  bf16
activations/weights, fp32 PSUM accumulation.
"""
import sys

sys.path.insert(0, "/opt/trn_rl_repo")

import numpy as np
import ml_dtypes

from concourse import bass, mybir, bacc, tile, bass_utils

bf16 = ml_dtypes.bfloat16

NCORES = 8
P = 128
D = 256
NUM_ENT = 100000
NUM_REL = 500
ENT_PAD = 100352            # 784 tiles
TILES = ENT_PAD // P        # 784
TPC = TILES // NCORES       # 98 tiles per core
RPC = ENT_PAD // NCORES     # 12544 rows per core
REL_PAD = 512
RT = REL_PAD // P           # 4 relation tiles

_prog_cache = {}


def _build_program(K, final, use_bias):
    """One conv layer (final=False) or conv2+final fused (final=True)."""
    C_total = int(sum(K))
    nc = bacc.Bacc("TRN2", target_bir_lowering=False, debug=False,
                   num_devices=NCORES)
    dt = mybir.dt
    T = lambda name, shape, d, kind: nc.dram_tensor(name, shape, d, kind=kind).ap()

    GATH = T("GATH", [P, C_total * D], dt.bfloat16, "ExternalInput")
    LR = T("LR", [P, C_total], dt.float32, "ExternalInput")
    NR = T("NR", [P, C_total], dt.float32, "ExternalInput")
    XPT = T("XPT", [P, 2, RPC], dt.bfloat16, "ExternalInput")
    XRPT = T("XRPT", [P, 2, REL_PAD], dt.bfloat16, "ExternalInput")
    WA = T("WA", [P, 2 * D], dt.bfloat16, "ExternalInput")
    WB = T("WB", [P, 2 * D], dt.bfloat16, "ExternalInput")
    WC = T("WC", [P, 2 * D], dt.bfloat16, "ExternalInput")
    WR = T("WR", [P, 2 * D], dt.bfloat16, "ExternalInput")
    EB = T("EB", [1, D], dt.bfloat16, "ExternalInput")
    RB = T("RB", [1, D], dt.bfloat16, "ExternalInput")
    IOTA = T("IOTA", [P, 2 * P], dt.bfloat16, "ExternalInput")
    ONES = T("ONES", [1, P], dt.bfloat16, "ExternalInput")
    if not final:
        XE_OUT = T("XE_OUT", [RPC, D], dt.bfloat16, "ExternalOutput")
        XR_OUT = T("XR_OUT", [REL_PAD, D], dt.bfloat16, "ExternalOutput")
    else:
        X0T = T("X0T", [P, 2, RPC], dt.bfloat16, "ExternalInput")
        X0RT = T("X0RT", [P, 2, REL_PAD], dt.bfloat16, "ExternalInput")
        WF0 = T("WF0", [P, 2 * D], dt.bfloat16, "ExternalInput")
        WF1 = T("WF1", [P, 2 * D], dt.bfloat16, "ExternalInput")
        WF2 = T("WF2", [P, 2 * D], dt.bfloat16, "ExternalInput")
        FB = T("FB", [1, D], dt.bfloat16, "ExternalInput")
        OUT_E = T("OUT_E", [RPC, D], dt.float32, "ExternalOutput")
        OUT_R = T("OUT_R", [REL_PAD, D], dt.float32, "ExternalOutput")

    off = np.concatenate([[0], np.cumsum(K)]).astype(int)
    Tanh = mybir.ActivationFunctionType.Tanh

    with tile.TileContext(nc) as tc:
        with (
            tc.tile_pool(name="consts", bufs=1) as cp,
            tc.tile_pool(name="work", bufs=3) as wp,
            tc.tile_pool(name="mpool", bufs=1) as mp,
            tc.tile_pool(name="psS", bufs=2, space="PSUM") as psS,
            tc.tile_pool(name="psE", bufs=2, space="PSUM") as psE,
            tc.tile_pool(name="psT", bufs=1, space="PSUM") as psT,
        ):
            # resident constants
            lr_sb = cp.tile([P, C_total], dt.float32, tag="lr")
            nc.sync.dma_start(lr_sb[:], LR[:])
            nr_sb = cp.tile([P, C_total], dt.float32, tag="nr")
            nc.sync.dma_start(nr_sb[:], NR[:])
            iota_sb = cp.tile([P, 2 * P], dt.bfloat16, tag="iota")
            nc.sync.dma_start(iota_sb[:], IOTA[:])
            ones_sb = cp.tile([1, P], dt.bfloat16, tag="ones")
            nc.sync.dma_start(ones_sb[:], ONES[:])

            def wtile(ap, name):
                t = cp.tile([P, 2 * D], dt.bfloat16, tag=f"w_{name}")
                nc.sync.dma_start(t[:], ap[:])
                return t

            wa, wb, wc, wr = (wtile(WA, "a"), wtile(WB, "b"),
                              wtile(WC, "c"), wtile(WR, "r"))

            def btile(ap, name):
                t = cp.tile([1, D], dt.bfloat16, tag=f"b_{name}")
                nc.sync.dma_start(t[:], ap[:])
                return t

            eb, rb = btile(EB, "e"), btile(RB, "r")
            if not use_bias:
                eb = rb = None
                fbN = None
            if final:
                wf0, wf1, wf2 = wtile(WF0, "f0"), wtile(WF1, "f1"), wtile(WF2, "f2")
                fb = btile(FB, "f") if use_bias else None

            def load_xt_pair(src, r0, n, tag, eng):
                xt = wp.tile([P, 2, n], dt.bfloat16, tag=tag)
                eng.dma_start(xt[:], src[:, :, r0:r0 + n])
                return xt

            def scatter_ST(t, g, ob):
                """Accumulate S^T for tile slot t from gathered chunk block g."""
                Kt = int(K[t])
                S0 = psS.tile([P, 2 * P], dt.float32, space="PSUM", tag="S0")
                S1 = psS.tile([P, 2 * P], dt.float32, space="PSUM", tag="S1")
                ms = []
                for c in range(Kt):
                    m = mp.tile([P, 2 * P], dt.bfloat16, tag=f"m{c % 8}")
                    oc = int(off[t]) + c
                    nc.vector.tensor_scalar(
                        out=m[:], in0=iota_sb[:],
                        scalar1=lr_sb[:, oc:oc + 1],
                        scalar2=nr_sb[:, oc:oc + 1],
                        op0=mybir.AluOpType.is_equal, op1=mybir.AluOpType.mult,
                    )
                    ms.append(m)
                for c in range(Kt):
                    cb = (ob + c) * D
                    nc.tensor.matmul(S0[:], lhsT=g[:, cb:cb + P], rhs=ms[c][:],
                                     start=(c == 0), stop=(c == Kt - 1))
                for c in range(Kt):
                    cb = (ob + c) * D
                    nc.tensor.matmul(S1[:], lhsT=g[:, cb + P:cb + D], rhs=ms[c][:],
                                     start=(c == 0), stop=(c == Kt - 1))
                sb0 = wp.tile([P, 2 * P], dt.bfloat16, tag="sb0")
                nc.vector.tensor_copy(sb0[:], S0[:])
                sb1 = wp.tile([P, 2 * P], dt.bfloat16, tag="sb1")
                nc.vector.tensor_copy(sb1[:], S1[:])
                return sb0, sb1

            def dense_row(terms, bias_sb, out_ap, r0, act2, eng_store):
                """Row-major dense: out[128r, 256] in one PSUM tile."""
                e = psE.tile([P, D], dt.float32, space="PSUM", tag="eout")
                mms = []
                for pair, w in terms:
                    mms.append((pair[0], w[:, 0:D]))
                    mms.append((pair[1], w[:, D:2 * D]))
                if bias_sb is not None:
                    mms.append((ones_sb[0:1, :], bias_sb[0:1, :]))
                for i, (l, r) in enumerate(mms):
                    nc.tensor.matmul(e[:], lhsT=l, rhs=r,
                                     start=(i == 0), stop=(i == len(mms) - 1))
                if act2:
                    t1 = wp.tile([P, D], dt.bfloat16, tag="t1")
                    nc.scalar.activation(t1[:], e[:], Tanh)
                    o = wp.tile([P, D], dt.bfloat16, tag="t2")
                    nc.scalar.activation(o[:], t1[:], Tanh)
                else:
                    o = wp.tile([P, D], dt.float32, tag="outf")
                    nc.vector.tensor_copy(o[:], e[:])
                eng_store.dma_start(out_ap[r0:r0 + P, :], o[:])
                return o

            def dense_T_tanh2(rhs_terms, bias_sb, x2t_tag):
                """Transposed dense -> x2^T [128f, 256r-of...]: out[:, j*128:] =
                tanh2( sum_k W[k,j].T-block @ rhs_k ).  rhs_terms: list of
                (w_tile, (rhs_k0, rhs_k1)); weight tile packed [128, 4*128]
                with block (2k+j)."""
                x2t = wp.tile([P, 2 * P], dt.bfloat16, tag=x2t_tag)
                for jj in (0, 1):
                    eT = psT.tile([P, P], dt.float32, space="PSUM", tag=f"eT{jj}")
                    mms = []
                    for w, rhs in rhs_terms:
                        for k in (0, 1):
                            blk = (2 * k + jj) * P
                            mms.append((w[:, blk:blk + P], rhs[k]))
                    if bias_sb is not None:
                        mms.append((bias_sb[0:1, jj * P:(jj + 1) * P], ones_sb[0:1, :]))
                    for i, (l, r) in enumerate(mms):
                        nc.tensor.matmul(eT[:], lhsT=l, rhs=r,
                                         start=(i == 0), stop=(i == len(mms) - 1))
                    ta = wp.tile([P, P], dt.bfloat16, tag=f"ta{jj}")
                    nc.scalar.activation(ta[:], eT[:], Tanh)
                    nc.scalar.activation(x2t[:, jj * P:(jj + 1) * P], ta[:], Tanh)
                return x2t

            # ================= entity tile pairs =================
            for pt in range(TPC // 2):
                t0 = 2 * pt
                o0, o2 = int(off[t0]), int(off[t0 + 2])
                g = wp.tile([P, (o2 - o0) * D], dt.bfloat16, tag="g")
                nc.sync.dma_start(g[:], GATH[:, o0 * D:o2 * D])
                xt = load_xt_pair(XPT, t0 * P, 2 * P, "xt", nc.scalar)
                if final:
                    xt0 = load_xt_pair(X0T, t0 * P, 2 * P, "xt0", nc.sync)
                for u in (0, 1):
                    t = t0 + u
                    usl = slice(u * P, (u + 1) * P)
                    sb0, sb1 = scatter_ST(t, g, int(off[t]) - o0)
                    if not final:
                        dense_row(
                            [((sb0[:, 0:P], sb1[:, 0:P]), wa),
                             ((xt[:, 0, usl], xt[:, 1, usl]), wb),
                             ((sb0[:, P:2 * P], sb1[:, P:2 * P]), wc)],
                            eb, XE_OUT, t * P, act2=True, eng_store=nc.gpsimd)
                    else:
                        x2t = dense_T_tanh2(
                            [(wa, (sb0[:, 0:P], sb1[:, 0:P])),
                             (wb, (xt[:, 0, usl], xt[:, 1, usl])),
                             (wc, (sb0[:, P:2 * P], sb1[:, P:2 * P]))],
                            eb, "x2t")
                        dense_row(
                            [((xt0[:, 0, usl], xt0[:, 1, usl]), wf0),
                             ((xt[:, 0, usl], xt[:, 1, usl]), wf1),
                             ((x2t[:, 0:P], x2t[:, P:2 * P]), wf2)],
                            fb, OUT_E, t * P, act2=False, eng_store=nc.gpsimd)

            # ================= relation tile pairs =================
            for pt in range(RT // 2):
                t0 = 2 * pt
                xtr = load_xt_pair(XRPT, t0 * P, 2 * P, "xtr", nc.scalar)
                if final:
                    xtr0 = load_xt_pair(X0RT, t0 * P, 2 * P, "xtr0", nc.sync)
                for u in (0, 1):
                    t = t0 + u
                    usl = slice(u * P, (u + 1) * P)
                    if not final:
                        dense_row(
                            [((xtr[:, 0, usl], xtr[:, 1, usl]), wr)],
                            rb, XR_OUT, t * P, act2=True, eng_store=nc.gpsimd)
                    else:
                        x2rt = dense_T_tanh2(
                            [(wr, (xtr[:, 0, usl], xtr[:, 1, usl]))], rb, "x2rt")
                        dense_row(
                            [((xtr0[:, 0, usl], xtr0[:, 1, usl]), wf0),
                             ((xtr[:, 0, usl], xtr[:, 1, usl]), wf1),
                             ((x2rt[:, 0:P], x2rt[:, P:2 * P]), wf2)],
                            fb, OUT_R, t * P, act2=False, eng_store=nc.gpsimd)
    nc.compile()
    return nc


def _get_program(K, final, use_bias):
    key = (tuple(K), final, use_bias)
    if key not in _prog_cache:
        _prog_cache[key] = _build_program(K, final, use_bias)
    return _prog_cache[key]


def _pack_w(w):
    """[256, 256] -> [128, 512]; [:, :256] = rows 0:128, [:, 256:] = rows 128:."""
    w = np.asarray(w, np.float32)
    return w.reshape(2, P, D).transpose(1, 0, 2).reshape(P, 2 * D).astype(bf16)


def _pack_wT(w):
    """[256, 256] -> [128, 4*128] blocks (2k+j) = W[k*128:(k+1)*128, j*128:(j+1)*128]."""
    w = np.asarray(w, np.float32)
    return np.ascontiguousarray(
        w.reshape(2, P, 2, P).transpose(1, 0, 2, 3).reshape(P, 4 * P)).astype(bf16)


def _transpose_feats(x):
    """[N, 256] -> [128, 2, N] with [p, k, r] = x[r, k*128+p]."""
    n = x.shape[0]
    return np.ascontiguousarray(np.asarray(x).reshape(n, 2, P).transpose(2, 1, 0))


def _inv_sqrt_deg(idx, n):
    deg = np.bincount(idx, minlength=n).astype(np.float32)
    with np.errstate(divide="ignore"):
        return np.where(deg > 0, deg ** -0.5, 0.0).astype(np.float32)


def kernel(x, edge_index, num_ent, num_rel, params):
    num_ent = int(num_ent)
    num_rel = int(num_rel)
    assert num_ent == NUM_ENT and num_rel == NUM_REL
    x = np.asarray(x, np.float32)
    ei = np.asarray(edge_index).astype(np.int64)
    E = ei.shape[1]
    a_, b_ = E // 3, (2 * E) // 3
    num_nodes = num_ent + num_rel

    # --- per-edge norms (opt_e and ipt_e modes only; r-modes are dead) ---
    rows = np.concatenate([ei[0, :a_], ei[0, a_:b_]])
    cols = np.concatenate([ei[1, :a_], ei[1, a_:b_]])
    nrms = []
    for lo, hi in ((0, a_), (a_, b_)):
        r, c = ei[0, lo:hi], ei[1, lo:hi]
        nrms.append(_inv_sqrt_deg(r, num_ent)[r] * _inv_sqrt_deg(c, num_nodes)[c])
    nrm = np.concatenate(nrms)
    is_ipt = np.zeros(rows.shape[0], np.int64)
    is_ipt[a_:] = 1

    # --- pack rows into tiles (snake deal by degree -> ~equal edges/tile) ---
    deg = np.bincount(rows, minlength=ENT_PAD)
    srt = np.argsort(-deg, kind="stable")
    snake = srt.reshape(P, TILES).copy()        # [round p, bin]
    snake[1::2] = snake[1::2, ::-1]
    bin_cnt = deg[snake].sum(0)
    brank = np.argsort(-bin_cnt, kind="stable")  # bins sorted by load desc
    # bin with rank r -> core r % 8, slot r // 8; device row = snake[p, bin]
    perm = np.empty(ENT_PAD, np.int64)           # device idx -> global row
    rr = np.arange(TILES)
    core_b = np.empty(TILES, np.int64); slot_b = np.empty(TILES, np.int64)
    core_b[brank] = rr % NCORES
    slot_b[brank] = rr // NCORES
    dev_base = core_b * RPC + slot_b * P         # per bin
    for p in range(P):
        perm[dev_base + p] = snake[p]
    inv = np.empty(ENT_PAD, np.int64)
    inv[perm] = np.arange(ENT_PAD)

    dev_idx = inv[rows]
    tile_dev = dev_idx // P                      # core-major tile id
    order = np.argsort(tile_dev, kind="stable")
    cols_s, nrm_s, ipt_s = cols[order], nrm[order], is_ipt[order]
    pp_s = (dev_idx % P)[order]
    cnt = np.bincount(tile_dev, minlength=TILES)
    # core-major: tile_dev = core * TPC + slot
    cnt_cs = cnt.reshape(NCORES, TPC)
    K = np.maximum(1, -(-cnt_cs.max(0) // P))
    off = np.concatenate([[0], np.cumsum(K)]).astype(int)
    C_total = int(off[-1])

    tstart = np.concatenate([[0], np.cumsum(cnt)]).astype(int)
    j = np.arange(dev_idx.shape[0]) - tstart[tile_dev[order]]

    lr_all = np.zeros((NCORES, C_total, P), np.float32)
    nr_all = np.zeros((NCORES, C_total, P), np.float32)
    col_all = np.zeros((NCORES, C_total, P), np.int64)
    core_s = tile_dev[order] // TPC
    slot_s = tile_dev[order] % TPC
    cpos = off[slot_s] + j // P
    ppos = j % P
    lr_all[core_s, cpos, ppos] = (pp_s + P * ipt_s).astype(np.float32)
    nr_all[core_s, cpos, ppos] = nrm_s
    col_all[core_s, cpos, ppos] = cols_s

    # --- folded weights ---
    layers = params["layers"]

    def fold(lp, packer):
        ew = np.asarray(lp["ent_w"], np.float32)
        rw = np.asarray(lp["rel_w"], np.float32)
        return dict(
            WA=packer(np.asarray(lp["w_opt_e"], np.float32) @ ew[0:D]),
            WB=packer(np.asarray(lp["w_loop_e"], np.float32) @ ew[D:2 * D]),
            WC=packer(np.asarray(lp["w_ipt_e"], np.float32) @ ew[2 * D:3 * D]),
            WR=packer(np.asarray(lp["w_loop_r"], np.float32) @ rw[D:2 * D]),
            EB=np.asarray(lp["ent_b"], np.float32).reshape(1, D).astype(bf16),
            RB=np.asarray(lp["rel_b"], np.float32).reshape(1, D).astype(bf16),
        )

    W1 = fold(layers[0], _pack_w)
    W2 = fold(layers[1], _pack_wT)          # launch B conv runs transposed
    Fw = np.asarray(params["final_w"], np.float32)
    WF = dict(WF0=_pack_w(Fw[0:D]), WF1=_pack_w(Fw[D:2 * D]),
              WF2=_pack_w(Fw[2 * D:3 * D]),
              FB=np.asarray(params["final_b"], np.float32).reshape(1, D).astype(bf16))

    bias_vals = [W1['EB'], W1['RB'], W2['EB'], W2['RB'], WF['FB']]
    iota = np.broadcast_to(np.arange(2 * P, dtype=np.float32), (P, 2 * P)).astype(bf16)
    ones = np.ones((1, P), bf16)

    x0e = np.zeros((ENT_PAD, D), bf16)
    x0e[:num_ent] = x[:num_ent].astype(bf16)
    x0r = np.zeros((REL_PAD, D), bf16)
    x0r[:num_rel] = x[num_ent:].astype(bf16)

    def gath(xsrc):
        g = xsrc[col_all]                      # [NCORES, C_total, P, D]
        return np.ascontiguousarray(g.transpose(0, 2, 1, 3)).reshape(NCORES, P, C_total * D)

    def in_maps(xe_prev_T, xr_prev_T, wts, g_all, extra=None):
        maps = []
        for k in range(NCORES):
            m = dict(
                GATH=g_all[k],
                LR=np.ascontiguousarray(lr_all[k].T),
                NR=np.ascontiguousarray(nr_all[k].T),
                XPT=xe_prev_T[k], XRPT=xr_prev_T,
                IOTA=iota, ONES=ones, **wts,
            )
            if extra:
                m.update({kk: (vv[k] if isinstance(vv, list) else vv)
                          for kk, vv in extra.items()})
            maps.append(m)
        return maps

    exec_ns = []

    def run(nc, maps, trace=False):
        res = bass_utils.run_bass_kernel_spmd(nc, maps, core_ids=list(range(NCORES)),
                                              trace=trace)
        if res.exec_time_ns:
            exec_ns.append(res.exec_time_ns)
        return res.results

    trace = bool(getattr(kernel, "trace", False))

    x0e_dev = x0e[perm]
    x0eT = [_transpose_feats(x0e_dev[k * RPC:(k + 1) * RPC]) for k in range(NCORES)]
    x0rT = _transpose_feats(x0r)

    # ---- launch A: conv layer 1 ----
    use_bias = bool(any(np.any(np.asarray(v, np.float32)) for v in bias_vals))
    ncA = _get_program(K, final=False, use_bias=use_bias)
    resA = run(ncA, in_maps(x0eT, x0rT, W1, gath(x0e)), trace)
    x1e_dev = np.concatenate([resA[k]["XE_OUT"] for k in range(NCORES)], 0)
    x1r = resA[0]["XR_OUT"]
    x1e = np.empty_like(x1e_dev)
    x1e[perm] = x1e_dev

    # ---- launch B: conv layer 2 + final projection (fused) ----
    ncB = _get_program(K, final=True, use_bias=use_bias)
    x1eT = [_transpose_feats(x1e_dev[k * RPC:(k + 1) * RPC]) for k in range(NCORES)]
    extra = {"X0T": x0eT, "X0RT": x0rT, **WF}
    resB = run(ncB, in_maps(x1eT, _transpose_feats(x1r), W2, gath(x1e), extra), trace)

    oe_dev = np.concatenate([resB[k]["OUT_E"] for k in range(NCORES)], 0)
    oe = np.empty_like(oe_dev)
    oe[perm] = oe_dev
    out = np.empty((num_nodes, D), np.float32)
    out[:num_ent] = oe[:num_ent]
    out[num_ent:] = resB[0]["OUT_R"][:num_rel]
    kernel.last_exec_ns = exec_ns
    return out
